# revision 1
# baseline (speedup 1.0000x reference)
"""NPS (non-printability score) kernel for Trainium2, 8-core data-parallel.

Math: for each pixel x (3 channels), distance to each of 30 printability
colors p_k is  d2_k = sum_c (x_c - p_c + 1e-6)^2 + 1e-6.  The score is
sum over pixels of sqrt(min_k d2_k), divided by adv_patch.size.

With q = p - 1e-6:  d2_k = S + (-2 x.q_k) + (T_k + 1e-6)  where
S = sum x_c^2, T_k = |q_k|^2.  For a block of 16 pixel "groups" the
TensorEngine computes d2 for 8 colors at a time via one block-diagonal
fp32 matmul over a 112-row feature vector per column (layouts chosen so
every engine operand starts on a 32-aligned partition window):
  rows  0..47  : x_c^2  (c*16+g)       weight 1
  rows 48..63  : ones                  weight T_k + 1e-6
  rows 64..111 : x_c    (64+c*16+g)    weight -2 q_c[k]
PSUM output partition (k*16+g) holds d2 of color k (of the pass) for
pixel group g.  A running DVE min over the 4 passes (one PSUM operand per
op  - a hardware rule), then a PE transpose + windowed free-dim reduce_min
collapse the 8 remaining colors (engines cannot shift partitions, so the
cross-partition min is done by transposing).  ScalarE does sqrt with a
fused per-partition sum; the per-core partials are combined on the host.

Sharding: batch dim (8 images) -> 8 NeuronCores, printability replicated.
"""

import numpy as np

import concourse.bass as bass
import concourse.bacc as bacc
import concourse.tile as tile
import concourse.mybir as mybir
from concourse.bass_utils import run_bass_kernel_spmd

F32 = mybir.dt.float32
I32 = mybir.dt.int32
ALU = mybir.AluOpType
ACTF = mybir.ActivationFunctionType

B, C, H, W = 8, 3, 512, 512
NCOLORS = 30
NPAD = 32            # colors padded to 32
NPASS = 4            # color passes, 8 colors each
CPP = 8              # colors per pass
G = 16               # pixel groups per matmul column block
MMN = 512            # matmul moving free dim (one fp32 PSUM bank)
HWPIX = H * W        # pixels per core (one image per core)
NFREE = 4096         # per-partition free size of one slab
NSLAB = HWPIX // (G * NFREE)   # 4
STS = NFREE // MMN   # supertiles per slab = 8
ST_TOT = NSLAB * STS  # 32
EPS = 1e-6


def _build_program(use_f32r=False, probe=None):
    nc = bacc.Bacc(
        "TRN2",
        target_bir_lowering=False,
        debug=False,
        enable_asserts=False,
        num_devices=B,
    )
    x_d = nc.dram_tensor("x", [NSLAB, C * G, NFREE], F32, kind="ExternalInput")
    p_d = nc.dram_tensor("p", [NCOLORS, C], F32, kind="ExternalInput")
    out_d = nc.dram_tensor("out", [128, ST_TOT], F32, kind="ExternalOutput")

    mm_dt = mybir.dt.float32r if use_f32r else F32

    with tile.TileContext(nc) as tc:
        _body(tc, nc, x_d, p_d, out_d, mm_dt, probe)
    nc.compile()
    return nc


def _body(tc, nc, x_d, p_d, out_d, mm_dt, probe=None):
    import contextlib

    ctx = contextlib.ExitStack()
    const = ctx.enter_context(tc.tile_pool(name="const", bufs=1))
    spool = ctx.enter_context(tc.tile_pool(name="spool", bufs=3))
    collp = ctx.enter_context(tc.tile_pool(name="collp", bufs=2))
    sqp = ctx.enter_context(tc.tile_pool(name="sqp", bufs=2))
    zpool = ctx.enter_context(tc.tile_pool(name="zpool", bufs=5, space="PSUM"))
    ptpool = ctx.enter_context(tc.tile_pool(name="ptpool", bufs=3, space="PSUM"))

    # ---------------- preamble: constants -------------------------------
    # register scalar constants used as activation biases
    for cval in (0.0, -EPS):
        ctile = const.tile([128, 1], F32, tag=f"const_{cval}")
        nc.vector.memset(ctile, cval)
        nc.const_aps.aps[(F32, cval)] = ctile[:]

    # tiny dummy activation: forces the ACT table load at t=0 instead of
    # serializing it behind the printability DMA
    warm = const.tile([1, 1], F32)
    nc.vector.memset(warm, 0.0)
    nc.scalar.activation(out=warm, in_=warm, func=ACTF.Square)

    # identity 128x128 for PE transpose; stencil112[p, c] = (p % 16 == c)
    iop128 = const.tile([128, 1], I32)
    nc.gpsimd.iota(iop128, pattern=[[0, 1]], base=0, channel_multiplier=1)
    iof128 = const.tile([128, 128], I32)
    nc.gpsimd.iota(iof128, pattern=[[1, 128]], base=0, channel_multiplier=0)
    id128 = const.tile([128, 128], mybir.dt.float16)
    nc.vector.tensor_tensor(
        out=id128, in0=iof128, in1=iop128.to_broadcast([128, 128]), op=ALU.is_equal
    )

    iop112 = const.tile([112, 1], I32)
    nc.gpsimd.iota(iop112, pattern=[[0, 1]], base=0, channel_multiplier=1)
    pm112 = const.tile([112, 1], I32)
    nc.vector.tensor_scalar(
        out=pm112, in0=iop112, scalar1=15, scalar2=None, op0=ALU.bitwise_and
    )
    iof112 = const.tile([112, 16], I32)
    nc.gpsimd.iota(iof112, pattern=[[1, 16]], base=0, channel_multiplier=0)
    sten = const.tile([112, 16], F32)
    nc.vector.tensor_tensor(
        out=sten, in0=iof112, in1=pm112.to_broadcast([112, 16]), op=ALU.is_equal
    )

    # ---------------- preamble: weight table ----------------------------
    # psbt[0, c, k] = printability[k, c]
    psbt = const.tile([1, C, NCOLORS], F32)
    with tc.high_priority():
        nc.sync.dma_start(out=psbt, in_=p_d.ap().transpose([1, 0]).unsqueeze(0))

    # W_flat[0, f*32 + k]: f 0-2 -> 1.0 (x^2 weights), f 3 -> T_k + eps,
    # f 4-6 -> -2 q_c[k] = -2 p + 2e-6
    wflat = const.tile([1, 7, NPAD], F32)
    nc.vector.memset(wflat, 0.0)
    nc.vector.memset(wflat[:, 0:3, :], 1.0)
    nc.scalar.activation(
        out=wflat[:, 4:7, 0:NCOLORS], in_=psbt, func=ACTF.Copy,
        bias=2.0 * EPS, scale=-2.0,
    )
    q2 = const.tile([1, C, NCOLORS], F32)
    nc.scalar.activation(out=q2, in_=psbt, func=ACTF.Square, bias=-EPS, scale=1.0)
    tsum = const.tile([1, NCOLORS], F32)
    nc.vector.tensor_add(out=tsum, in0=q2[:, 0, :], in1=q2[:, 1, :])
    nc.vector.scalar_tensor_tensor(
        out=wflat[:, 3, 0:NCOLORS], in0=tsum, scalar=EPS, in1=q2[:, 2, :],
        op0=ALU.add, op1=ALU.add,
    )
    # padded colors: huge constant term so they never win the min
    nc.vector.memset(wflat[:, 3, NCOLORS:NPAD], 1.0e9)

    # broadcast each feature row to its 16-partition block:
    # wbc[16f+g, k] = W[f, k].  partition_broadcast gives every partition
    # the whole table; 7 masked copies then select partition-block f.
    wbig = const.tile([112, 7 * NPAD], F32)
    nc.gpsimd.partition_broadcast(wbig, wflat.rearrange("p f k -> p (f k)"))
    pdiv = const.tile([112, 1], I32)
    nc.vector.tensor_scalar(
        out=pdiv, in0=iop112, scalar1=4, scalar2=None, op0=ALU.arith_shift_right
    )
    wbc = const.tile([112, NPAD], F32)
    for f in range(7):
        mf = const.tile([112, 1], I32, tag=f"mf{f}")
        nc.vector.tensor_scalar(
            out=mf, in0=pdiv, scalar1=f, scalar2=None, op0=ALU.is_equal
        )
        nc.vector.copy_predicated(
            out=wbc,
            mask=mf.to_broadcast([112, NPAD]),
            data=wbig[:, f * NPAD:(f + 1) * NPAD],
        )

    # lhsT[p, 128j + k*16 + g] = sten[p, g] * wbc[p, 8j + k]
    lhsT = const.tile([112, NPASS * 128], mm_dt)
    for j in range(NPASS):
        outv = lhsT[:, 128 * j:128 * (j + 1)].rearrange("p (k g) -> p k g", g=G)
        in0 = sten.unsqueeze(1).to_broadcast([112, CPP, G])
        in1 = wbc[:, CPP * j:CPP * (j + 1)].unsqueeze(2).to_broadcast([112, CPP, G])
        nc.vector.tensor_tensor(out=outv, in0=in0, in1=in1, op=ALU.mult)

    # ---------------- rhs buffers (manual 2-buffer rotation) -------------
    # rows 0..47 squares, 48..63 ones, 64..111 x.  x is DMA'd twice: once
    # into a base-0 staging tile (ScalarE requires equal start partitions
    # for in/out, so Square must run 0->0) and once into rows 64..111.
    rhs_bufs = []
    xstage_bufs = []
    for i in range(3):
        t = const.tile([112, NFREE], mm_dt, tag=f"rhs{i}")
        # f32r memset is not in the ISA; 1.0f is exact in any rounding,
        # so write the bits through an f32 view (gpsimd: keep DVE free).
        # buf 0 is split so the first supertile's columns are ready early.
        if i == 0:
            nc.gpsimd.memset(t[32:64, 0:MMN].bitcast(F32), 1.0)
            nc.gpsimd.memset(t[32:64, MMN:].bitcast(F32), 1.0)
        else:
            nc.gpsimd.memset(t[32:64, :].bitcast(F32), 1.0)
        rhs_bufs.append(t)
        xst = const.tile([48, NFREE], F32, tag=f"xstage{i}")
        xstage_bufs.append(xst)

    acc = const.tile([128, ST_TOT], F32)
    if probe is not None:
        nc.vector.memset(acc, 0.0)

    # ---------------- main loop -----------------------------------------
    for s in range(NSLAB):
        rhs = rhs_bufs[s % 3]
        xstage = xstage_bufs[s % 3]
        # gpsimd DMA: the only engine that may cast (fp32 -> fp32r)
        xdma = nc.gpsimd.dma_start if mm_dt != F32 else nc.sync.dma_start
        if s == 0:
            # split the first slab's loads/squares so supertile 0 unblocks
            # the PE as early as possible
            nc.sync.dma_start(out=xstage[:, 0:MMN], in_=x_d.ap()[s][:, 0:MMN])
            xdma(out=rhs[64:112, 0:MMN], in_=x_d.ap()[s][:, 0:MMN])
            nc.scalar.activation(
                out=rhs[0:48, 0:MMN], in_=xstage[:, 0:MMN], func=ACTF.Square
            )
            nc.sync.dma_start(out=xstage[:, MMN:], in_=x_d.ap()[s][:, MMN:])
            xdma(out=rhs[64:112, MMN:], in_=x_d.ap()[s][:, MMN:])
            nc.scalar.activation(
                out=rhs[0:48, MMN:], in_=xstage[:, MMN:], func=ACTF.Square
            )
        else:
            nc.sync.dma_start(out=xstage, in_=x_d.ap()[s])
            xdma(out=rhs[64:112, :], in_=x_d.ap()[s])
            nc.scalar.activation(out=rhs[0:48, :], in_=xstage, func=ACTF.Square)
        for t in range(STS):
            st = s * STS + t
            rsl = rhs[0:112, t * MMN:(t + 1) * MMN]
            zs = []
            for j in range(NPASS):
                z = zpool.tile([128, MMN], F32, tag="z")
                nc.tensor.matmul(
                    out=z,
                    lhsT=lhsT[:, 128 * j:128 * (j + 1)],
                    rhs=rsl,
                    start=True,
                    stop=True,
                )
                zs.append(z)
            if probe == "pe_only":
                continue
            # running min over the 4 passes (TT may read only 1 PSUM input)
            stile = spool.tile([128, MMN], F32, tag="s")
            nc.scalar.copy(out=stile, in_=zs[0])
            nc.vector.tensor_tensor(out=stile, in0=stile, in1=zs[1], op=ALU.min)
            nc.vector.tensor_tensor(out=stile, in0=stile, in1=zs[2], op=ALU.min)
            # last min narrows to fp16: d2 rounding is relative (no
            # cancellation risk) and fp16 transposes run 2x on the PE
            st16 = spool.tile([128, MMN], mybir.dt.float16, tag="s16")
            nc.vector.tensor_tensor(out=st16, in0=stile, in1=zs[3], op=ALU.min)
            if probe == "no_transpose":
                continue
            # 8 colors left on partitions (k*16+g).  Engines cannot read
            # across partition windows, so transpose and reduce on free dim.
            pt = ptpool.tile([128, 4, 128], mybir.dt.float16, tag="pt")
            for ch in range(4):
                nc.tensor.transpose(
                    out=pt[:, ch, :],
                    in_=st16[:, 128 * ch:128 * (ch + 1)],
                    identity=id128,
                )
            coll = collp.tile([128, 4, 16], F32, tag="coll")
            ptv = pt.rearrange("p c (k g) -> p c g k", k=CPP)
            nc.vector.tensor_reduce(
                out=coll, in_=ptv, axis=mybir.AxisListType.X, op=ALU.min
            )
            sqt = sqp.tile([128, 64], F32, tag="sq")
            nc.scalar.activation(
                out=sqt,
                in_=coll.rearrange("p a b -> p (a b)"),
                func=ACTF.Sqrt,
                accum_out=acc[:, st:st + 1],
            )

    nc.sync.dma_start(out=out_d.ap(), in_=acc)
    ctx.close()


_CACHE = {}


def _get_program(use_f32r=False, probe=None):
    key = ("prog", use_f32r, probe)
    if key not in _CACHE:
        _CACHE[key] = _build_program(use_f32r, probe)
    return _CACHE[key]


def kernel(adv_patch: np.ndarray, printability: np.ndarray) -> np.ndarray:
    # device layout: [slab, (c,g), n] with pixel (s, g, n) = s*65536 + g*4096 + n
    x = np.ascontiguousarray(
        np.asarray(adv_patch, dtype=np.float32)
        .reshape(B, C, NSLAB, G, NFREE)
        .transpose(0, 2, 1, 3, 4)
    )
    p = np.ascontiguousarray(printability, dtype=np.float32)
    nc = _get_program()
    in_maps = [{"x": x[b], "p": p} for b in range(B)]
    res = run_bass_kernel_spmd(nc, in_maps, core_ids=list(range(B)))
    total = np.float64(0.0)
    for r in res.results:
        total += r["out"].astype(np.float64).sum()
    return np.float32(total / (B * C * H * W))


def profile_once(inputs, trace_cores=None):
    """Run once with NTFF tracing; return max per-core exec_time_ns or None."""
    x = np.ascontiguousarray(
        np.asarray(inputs["adv_patch"], dtype=np.float32)
        .reshape(B, C, NSLAB, G, NFREE)
        .transpose(0, 2, 1, 3, 4)
    )
    p = np.ascontiguousarray(inputs["printability"], dtype=np.float32)
    nc = _get_program()
    in_maps = [{"x": x[b], "p": p} for b in range(B)]
    try:
        res = run_bass_kernel_spmd(
            nc,
            in_maps,
            core_ids=list(range(B)),
            trace=True,
            trace_cores=trace_cores,
        )
        if res.instructions_and_trace is not None:
            print("trace:", res.instructions_and_trace[1])
        return res.exec_time_ns
    except Exception as e:  # profiling is best-effort
        print("profile_once failed:", e)
        return None



# revision 15
# speedup vs baseline: 1.4394x; 1.4394x over previous
"""NPS (non-printability score) kernel for Trainium2, 8-core data-parallel.

Math: for each pixel x (3 channels), distance to each of 30 printability
colors p_k is  d2_k = sum_c (x_c - p_c + 1e-6)^2 + 1e-6.  The score is
sum over pixels of sqrt(min_k d2_k), divided by adv_patch.size.

With q = p - 1e-6:  d2_k = S + (-2 x.q_k) + (T_k + 1e-6) where S = sum
x_c^2, T_k = |q_k|^2.  fp16 matmuls (1 PE cycle/column vs fp32's 4)
compute d2 for 8 colors x 16 pixel groups per 512-column pass; 4 passes
cover the 32 (padded) colors.  T rides in as two "ones"-row weights
(split hi/lo over two fp16 rows to kill weight-rounding error); the
ones rows and x rows arrive in a single HBM load per slab.

Post-matmul min funnel (z is fp32 in PSUM - TRN2 matmuls can't narrow -
so every z value crosses PSUM->SBUF exactly once, spread over engines):
  - ScalarE: passes 0,1 -> fp16 SBUF via Relu (per-slot ops on a 3-bank
    rotation so PSUM frees early; Relu also clamps rounding negatives).
  - DVE: fp16 min of the two converted passes (packed fp16 = 2x rate),
    then a chained min against pass 2 read straight from PSUM.
  - Pass 3 is DMA-drained to SBUF so GPSIMD - which cannot touch PSUM -
    applies the last pass-min.
  - PE transposes the survivor (fp16 2x; colors k are the innermost 8
    of each transposed block since lhsT columns are g*8+k), then one
    DVE windowed reduce_min folds the colors; per-pixel minima collect
    into a [128, 8*128] tile per 8 pairs; a tensor_scalar max-0 clamp
    (fp16 rounding can push min-d2 slightly negative) and one ScalarE
    sqrt+accumulate per collector finish the job.

PSUM (8 banks): z01 rotation 3 + z2 [128,2,512] 2 + z3 stage 2 + pt 1.

Sharding: batch dim (8 images) -> 8 NeuronCores, printability replicated.
"""

import numpy as np

import concourse.bass as bass
import concourse.bacc as bacc
import concourse.tile as tile
import concourse.mybir as mybir
from concourse.bass_utils import run_bass_kernel_spmd

F32 = mybir.dt.float32
F16 = mybir.dt.float16
I32 = mybir.dt.int32
ALU = mybir.AluOpType
ACTF = mybir.ActivationFunctionType

B, C, H, W = 8, 3, 512, 512
NCOLORS = 30
NPAD = 32            # colors padded to 32
NPASS = 4            # color passes, 8 colors each
CPP = 8              # colors per pass
G = 16               # pixel groups per matmul column block
MMN = 512            # matmul moving free dim (one fp32 PSUM bank)
NFREE = 4096         # per-partition free size of one slab
NSLAB = 4            # 4 slabs x 16 groups x 4096 = 262144 pixels/core
STS = NFREE // MMN   # supertiles per slab = 8
NPAIR = NSLAB * STS // 2   # 16 parity pairs
X0 = 64              # x rows base in rhs (rows 50..63 are zero pad)
ONES0 = 48           # two ones rows (T_hi / T_lo weights)
ROWS = 112
EPS = 1e-6
TBIG = 60000.0       # padded-color T: huge but finite in fp16


def _build_program(probe=None):
    nc = bacc.Bacc(
        "TRN2",
        target_bir_lowering=False,
        debug=False,
        enable_asserts=False,
        num_devices=B,
    )
    # x slab rows: 0..1 ones, 2..15 zero pad, 16..63 x (c*16+g)
    x_d = nc.dram_tensor("x", [NSLAB, 64, NFREE], F16, kind="ExternalInput")
    p_d = nc.dram_tensor("p", [NCOLORS, C], F32, kind="ExternalInput")
    out_d = nc.dram_tensor("out", [128, 2], F32, kind="ExternalOutput")

    with tile.TileContext(nc) as tc:
        _body(tc, nc, x_d, p_d, out_d, probe)
    nc.compile()
    return nc


def _body(tc, nc, x_d, p_d, out_d, probe=None):
    import contextlib

    ctx = contextlib.ExitStack()
    const = ctx.enter_context(tc.tile_pool(name="const", bufs=1))
    spool = ctx.enter_context(tc.tile_pool(name="spool", bufs=3))
    mpool = ctx.enter_context(tc.tile_pool(name="mpool", bufs=3))
    cpool = ctx.enter_context(tc.tile_pool(name="cpool", bufs=3))
    s2pool = ctx.enter_context(tc.tile_pool(name="s2pool", bufs=3))
    stpool = ctx.enter_context(tc.tile_pool(name="stpool", bufs=4))
    collp = ctx.enter_context(tc.tile_pool(name="collp", bufs=2))
    sqp = ctx.enter_context(tc.tile_pool(name="sqp", bufs=2))
    zpool = ctx.enter_context(tc.tile_pool(name="zpool", bufs=3, space="PSUM"))
    z2pool = ctx.enter_context(tc.tile_pool(name="z2pool", bufs=1, space="PSUM"))
    z3pool = ctx.enter_context(tc.tile_pool(name="z3pool", bufs=1, space="PSUM"))
    ptpool = ctx.enter_context(tc.tile_pool(name="ptpool", bufs=1, space="PSUM"))

    # ---------------- preamble: constants -------------------------------
    czero = const.tile([128, 1], F32)
    nc.vector.memset(czero, 0.0)
    nc.const_aps.aps[(F32, 0.0)] = czero[:]

    psbt = const.tile([1, C, NCOLORS], F32)
    hp = tc.high_priority()
    hp.__enter__()
    nc.sync.dma_start(out=psbt, in_=p_d.ap().transpose([1, 0]).unsqueeze(0))
    # rhs buffers: rows 0..47 squares, 48..49 ones + 50..63 pad (row 52
    # carries the fp16 printability table in slab 0) + 64..111 x
    rhs_bufs = []
    for i in range(3):
        rhs = const.tile([ROWS, NFREE], F16, tag=f"rhs{i}")
        rhs_bufs.append(rhs)
    for s in range(NSLAB):
        eng = nc.sync if s % 2 == 0 else nc.scalar
        eng.dma_start(out=rhs_bufs[s % 3][ONES0:ROWS, :], in_=x_d.ap()[s])


    # identity 128x128 fp16 for PE transpose
    iop128 = const.tile([128, 1], I32)
    nc.gpsimd.iota(iop128, pattern=[[0, 1]], base=0, channel_multiplier=1)
    iof128 = const.tile([128, 128], I32)
    nc.gpsimd.iota(iof128, pattern=[[1, 128]], base=0, channel_multiplier=0)
    id128 = const.tile([128, 128], F16)
    nc.vector.tensor_tensor(
        out=id128, in0=iof128, in1=iop128.to_broadcast([128, 128]), op=ALU.is_equal
    )

    # sten[p, g] = ((p & 15) == g); ones rows 48..49 forced to 1 for all g
    iop112 = const.tile([ROWS, 1], I32)
    nc.gpsimd.iota(iop112, pattern=[[0, 1]], base=0, channel_multiplier=1)
    pm112 = const.tile([ROWS, 1], I32)
    nc.vector.tensor_scalar(
        out=pm112, in0=iop112, scalar1=15, scalar2=None, op0=ALU.bitwise_and
    )
    iof16 = const.tile([ROWS, G], I32)
    nc.gpsimd.iota(iof16, pattern=[[1, G]], base=0, channel_multiplier=0)
    sten = const.tile([ROWS, G], F32)
    nc.vector.tensor_tensor(
        out=sten, in0=iof16, in1=pm112.to_broadcast([ROWS, G]), op=ALU.is_equal
    )
    mo_ge = const.tile([ROWS, 1], I32)
    nc.vector.tensor_scalar(out=mo_ge, in0=iop112, scalar1=ONES0 - 1,
                            scalar2=None, op0=ALU.is_gt)
    mo_lt = const.tile([ROWS, 1], I32)
    nc.vector.tensor_scalar(out=mo_lt, in0=iop112, scalar1=ONES0 + 2,
                            scalar2=None, op0=ALU.is_lt)
    mo = const.tile([ROWS, 1], F32)
    nc.vector.tensor_tensor(out=mo, in0=mo_ge, in1=mo_lt, op=ALU.mult)
    nc.vector.tensor_tensor(out=sten, in0=sten,
                            in1=mo.to_broadcast([ROWS, G]), op=ALU.max)

    # ---------------- preamble: weight table ----------------------------
    # q = p - eps;  T_k = |q_k|^2 + eps  (split hi/lo over two fp16 rows)
    qt = const.tile([1, C, NCOLORS], F32)
    nc.vector.tensor_scalar(out=qt, in0=psbt, scalar1=-EPS, scalar2=None,
                            op0=ALU.add)
    q2 = const.tile([1, C, NCOLORS], F32)
    nc.vector.tensor_tensor(out=q2, in0=qt, in1=qt, op=ALU.mult)
    tsum = const.tile([1, NCOLORS], F32)
    nc.vector.tensor_add(out=tsum, in0=q2[:, 0, :], in1=q2[:, 1, :])
    t32 = const.tile([1, NPAD], F32)
    nc.vector.memset(t32, TBIG)
    nc.vector.scalar_tensor_tensor(
        out=t32[:, 0:NCOLORS], in0=tsum, scalar=EPS, in1=q2[:, 2, :],
        op0=ALU.add, op1=ALU.add,
    )
    thi16 = const.tile([1, NPAD], F16)
    nc.vector.tensor_scalar(out=thi16, in0=t32, scalar1=1.0, scalar2=None,
                            op0=ALU.mult)
    thi32 = const.tile([1, NPAD], F32)
    nc.vector.tensor_scalar(out=thi32, in0=thi16, scalar1=1.0, scalar2=None,
                            op0=ALU.mult)
    tlo32 = const.tile([1, NPAD], F32)
    nc.vector.tensor_tensor(out=tlo32, in0=t32, in1=thi32, op=ALU.subtract)

    # wtab [1, 6, 32]: b0 = 1.0 (x^2 rows), b1 = T_hi, b2 = T_lo,
    # b3..b5 = -2 q_c  (padded colors: T = TBIG, q = 0)
    wtab = const.tile([1, 6, NPAD], F32)
    nc.vector.memset(wtab, 0.0)
    nc.vector.memset(wtab[:, 0, :], 1.0)
    nc.vector.tensor_scalar(out=wtab[:, 1, :], in0=thi32, scalar1=1.0,
                            scalar2=None, op0=ALU.mult)
    nc.vector.tensor_scalar(out=wtab[:, 2, :], in0=tlo32, scalar1=1.0,
                            scalar2=None, op0=ALU.mult)
    nc.vector.tensor_scalar(out=wtab[:, 3:6, 0:NCOLORS], in0=qt, scalar1=-2.0,
                            scalar2=None, op0=ALU.mult)

    # broadcast to all partitions, then per-partition-block select
    wbig = const.tile([ROWS, 6 * NPAD], F32)
    nc.gpsimd.partition_broadcast(wbig, wtab.rearrange("p f k -> p (f k)"))
    wsel = const.tile([ROWS, NPAD], F32)
    nc.vector.memset(wsel, 0.0)
    for blk, (lo, hi) in enumerate(
        [(0, 48), (48, 49), (49, 50), (64, 80), (80, 96), (96, 112)]
    ):
        mge = const.tile([ROWS, 1], I32, tag=f"mge{blk}")
        nc.vector.tensor_scalar(
            out=mge, in0=iop112, scalar1=lo - 1, scalar2=None, op0=ALU.is_gt
        )
        mlt = const.tile([ROWS, 1], I32, tag=f"mlt{blk}")
        nc.vector.tensor_scalar(
            out=mlt, in0=iop112, scalar1=hi, scalar2=None, op0=ALU.is_lt
        )
        mm = const.tile([ROWS, 1], I32, tag=f"mm{blk}")
        nc.vector.tensor_tensor(out=mm, in0=mge, in1=mlt, op=ALU.mult)
        nc.vector.copy_predicated(
            out=wsel,
            mask=mm.to_broadcast([ROWS, NPAD]),
            data=wbig[:, blk * NPAD:(blk + 1) * NPAD],
        )

    # lhsT[p, 128j + g*8 + k] = sten[p, g] * wsel[p, 8j + k]   (k minor!)
    lhsT = const.tile([ROWS, NPASS * 128], F16)
    for j in range(NPASS):
        outv = lhsT[:, 128 * j:128 * (j + 1)].rearrange("p (g k) -> p g k", k=CPP)
        in0 = sten.unsqueeze(2).to_broadcast([ROWS, G, CPP])
        in1 = wsel[:, CPP * j:CPP * (j + 1)].unsqueeze(1).to_broadcast(
            [ROWS, G, CPP])
        nc.vector.tensor_tensor(out=outv, in0=in0, in1=in1, op=ALU.mult)
    hp.__exit__(None, None, None)

    # squares all on GPSIMD (Multiply is in its supported op set),
    # in quarters so nothing blocks the Pool stream for long
    def emit_square(s, quarter=None):
        rhs = rhs_bufs[s % 3]
        qs = [quarter] if quarter is not None else range(4)
        for q in qs:
            q0 = q * (NFREE // 4)
            sl = slice(q0, q0 + NFREE // 4)
            nc.gpsimd.tensor_tensor(
                out=rhs[0:48, sl], in0=rhs[X0:ROWS, sl], in1=rhs[X0:ROWS, sl],
                op=ALU.mult,
            )

    emit_square(0)
    emit_square(1)

    # PE p-state warm-up: harmless matmuls on the weight tile ramp the
    # tensor engine to full clock while the first slab loads
    for _ in range(12):
        zw = zpool.tile([128, MMN], F32, tag="z01")
        nc.tensor.matmul(out=zw, lhsT=lhsT[:, 0:128], rhs=lhsT[:, 0:MMN],
                         start=True, stop=True)

    z2 = z2pool.tile([128, 2, MMN], F32)       # pass 2, parity slots
    z3 = z3pool.tile([128, 2, MMN], F32)       # pass 3, parity slots
    pt = ptpool.tile([128, 2, 4, 128], F16)    # transposed survivors

    acc = const.tile([128, 2], F32)
    if probe is not None:
        nc.vector.memset(acc, 0.0)

    collectors = []

    def emit_color_min(pair, stile_of):
        # transposes (PE) for `pair`, then fold the packed 8 colors (DVE)
        stile = stile_of[pair]
        for par in range(2):
            for chb in range(4):
                nc.tensor.transpose(
                    out=pt[:, par, chb, :],
                    in_=stile[:, par, 128 * chb:128 * (chb + 1)],
                    identity=id128,
                )
        if pair % 8 == 0:
            coll_new = collp.tile([128, 8, 128], F16, tag="coll")
            collectors.append(coll_new)
        coll = collectors[-1]
        ptv = pt.rearrange("p q c (g k) -> p q c g k", k=CPP)
        outv = coll[:, pair % 8, :].rearrange("p (q c g) -> p q c g", q=2, c=4)
        nc.vector.tensor_reduce(
            out=outv, in_=ptv, axis=mybir.AxisListType.X, op=ALU.min
        )

    def emit_collector_finish(r):
        coll = collectors[r]
        nc.vector.tensor_scalar(
            out=coll, in0=coll, scalar1=0.0, scalar2=None, op0=ALU.max
        )
        scratch = sqp.tile([128, 8 * 128], F16, tag="sq")
        nc.scalar.activation(
            out=scratch, in_=coll.rearrange("p a b -> p (a b)"),
            func=ACTF.Sqrt, accum_out=acc[:, r:r + 1],
        )

    # ---------------- main loop -----------------------------------------
    stile_of = {}
    for pair in range(NPAIR):
        slab = pair // 4
        rhs = rhs_bufs[slab % 3]
        s16 = spool.tile([128, 2, 2, MMN], F16, tag="s16")
        for par in range(2):
            st = pair * 2 + par
            t = st % STS
            rsl = rhs[:, t * MMN:(t + 1) * MMN]
            for j in range(NPASS):
                if j < 2:
                    zt = zpool.tile([128, MMN], F32, tag="z01")
                elif j == 2:
                    zt = z2[:, par, :]
                else:
                    zt = z3[:, par, :]
                nc.tensor.matmul(
                    out=zt,
                    lhsT=lhsT[:, 128 * j:128 * (j + 1)],
                    rhs=rsl,
                    start=True,
                    stop=True,
                )
                if j < 2:
                    nc.scalar.activation(
                        out=s16[:, par, j, :], in_=zt, func=ACTF.Relu
                    )
        # transposes + color fold of an older pair ride here so the PE
        # never waits on the (deep) min pipeline
        if probe != "pe_only" and pair >= 3:
            emit_color_min(pair - 3, stile_of)
            stile_of.pop(pair - 3)

        if probe == "pe_only":
            continue

        # ScalarE also converts pass 2 (pair-merged); DVE folds the three
        # converted passes at fp16 2x and chains pass 3 from PSUM
        s2 = s2pool.tile([128, 2, MMN], F16, tag="s2")
        nc.scalar.activation(out=s2, in_=z2, func=ACTF.Relu)
        m1 = mpool.tile([128, 2, MMN], F16, tag="m1")
        nc.vector.tensor_tensor(
            out=m1, in0=s16[:, :, 0, :], in1=s16[:, :, 1, :], op=ALU.min
        )
        m2 = cpool.tile([128, 2, MMN], F16, tag="m2")
        nc.vector.tensor_tensor(out=m2, in0=m1, in1=s2, op=ALU.min)
        stile = stpool.tile([128, 2, MMN], F16, tag="stile")
        nc.vector.tensor_tensor(out=stile, in0=m2, in1=z3, op=ALU.min)
        stile_of[pair] = stile

        if 3 <= pair <= 6:
            emit_square(2, quarter=pair - 3)
        elif 7 <= pair <= 10:
            emit_square(3, quarter=pair - 7)


    if probe != "pe_only":
        for p in (NPAIR - 3, NPAIR - 2, NPAIR - 1):
            emit_color_min(p, stile_of)
        emit_collector_finish(0)
        emit_collector_finish(1)

    nc.sync.dma_start(out=out_d.ap(), in_=acc)
    ctx.close()


_CACHE = {}


def _get_program(probe=None):
    key = ("prog", probe)
    if key not in _CACHE:
        _CACHE[key] = _build_program(probe)
    return _CACHE[key]


def _prep_x(adv_patch):
    # device layout per slab: rows 0..1 ones, 2..15 zero, 16..63 x(c*16+g)
    x = (
        np.asarray(adv_patch, dtype=np.float32)
        .reshape(B, C, NSLAB, G, NFREE)
        .transpose(0, 2, 1, 3, 4)
        .reshape(B, NSLAB, 48, NFREE)
        .astype(np.float16)
    )
    xd = np.zeros((B, NSLAB, 64, NFREE), dtype=np.float16)
    xd[:, :, 0:2, :] = np.float16(1.0)
    xd[:, :, 16:64, :] = x
    return np.ascontiguousarray(xd)


def kernel(adv_patch: np.ndarray, printability: np.ndarray) -> np.ndarray:
    xd = _prep_x(adv_patch)
    p = np.ascontiguousarray(printability, dtype=np.float32)
    nc = _get_program()
    in_maps = [{"x": xd[b], "p": p} for b in range(B)]
    res = run_bass_kernel_spmd(nc, in_maps, core_ids=list(range(B)))
    total = np.float64(0.0)
    for r in res.results:
        total += r["out"].astype(np.float64).sum()
    return np.float32(total / (B * C * H * W))


def profile_once(inputs, trace_cores=None):
    xd = _prep_x(inputs["adv_patch"])
    p = np.ascontiguousarray(inputs["printability"], dtype=np.float32)
    nc = _get_program()
    in_maps = [{"x": xd[b], "p": p} for b in range(B)]
    try:
        res = run_bass_kernel_spmd(
            nc, in_maps, core_ids=list(range(B)), trace=True,
            trace_cores=trace_cores,
        )
        if res.instructions_and_trace is not None:
            print("trace:", res.instructions_and_trace[1])
        return res.exec_time_ns
    except Exception as e:
        print("profile_once failed:", e)
        return None


# revision 19
# speedup vs baseline: 1.6633x; 1.1555x over previous
"""NPS (non-printability score) kernel for Trainium2, 8-core data-parallel.

Math: for each pixel x (3 channels), distance to each of 30 printability
colors p_k is  d2_k = sum_c (x_c - p_c + 1e-6)^2 + 1e-6.  The score is
sum over pixels of sqrt(min_k d2_k), divided by adv_patch.size.

With q = p - 1e-6:  d2_k = S + (-2 x.q_k) + (T_k + 1e-6) where S = sum
x_c^2, T_k = |q_k|^2.  fp16 matmuls (1 PE cycle/column vs fp32's 4)
compute d2 for 8 colors x 16 pixel groups per 512-column pass; 4 passes
cover the 32 (padded) colors.  T rides in as two "ones"-row weights
(split hi/lo over two fp16 rows to kill weight-rounding error); the
ones rows and x rows arrive in a single HBM load per slab.

Post-matmul min funnel (z is fp32 in PSUM - TRN2 matmuls can't narrow -
so every z value crosses PSUM->SBUF exactly once, spread over engines):
  - ScalarE: passes 0,1 -> fp16 SBUF via Relu (per-slot ops on a 3-bank
    rotation so PSUM frees early; Relu also clamps rounding negatives).
  - DVE: fp16 min of the two converted passes (packed fp16 = 2x rate),
    then a chained min against pass 2 read straight from PSUM.
  - Pass 3 is DMA-drained to SBUF so GPSIMD - which cannot touch PSUM -
    applies the last pass-min.
  - PE transposes the survivor (fp16 2x; colors k are the innermost 8
    of each transposed block since lhsT columns are g*8+k), then one
    DVE windowed reduce_min folds the colors; per-pixel minima collect
    into a [128, 8*128] tile per 8 pairs; a tensor_scalar max-0 clamp
    (fp16 rounding can push min-d2 slightly negative) and one ScalarE
    sqrt+accumulate per collector finish the job.

PSUM (8 banks): z01 rotation 3 + z2 [128,2,512] 2 + z3 stage 2 + pt 1.

Sharding: batch dim (8 images) -> 8 NeuronCores, printability replicated.
"""

import numpy as np

import concourse.bass as bass
import concourse.bacc as bacc
import concourse.tile as tile
import concourse.mybir as mybir
from concourse.bass_utils import run_bass_kernel_spmd

F32 = mybir.dt.float32
F16 = mybir.dt.float16
I32 = mybir.dt.int32
ALU = mybir.AluOpType
ACTF = mybir.ActivationFunctionType

B, C, H, W = 8, 3, 512, 512
NCOLORS = 30
NPAD = 32            # colors padded to 32
NPASS = 4            # color passes, 8 colors each
CPP = 8              # colors per pass
G = 16               # pixel groups per matmul column block
MMN = 512            # matmul moving free dim (one fp32 PSUM bank)
NFREE = 4096         # per-partition free size of one slab
NSLAB = 4            # 4 slabs x 16 groups x 4096 = 262144 pixels/core
STS = NFREE // MMN   # supertiles per slab = 8
NPAIR = NSLAB * STS // 2   # 16 parity pairs
X0 = 64              # x rows base in rhs (rows 50..63 are zero pad)
ONES0 = 48           # two ones rows (T_hi / T_lo weights)
ROWS = 112
EPS = 1e-6
TBIG = 60000.0       # padded-color T: huge but finite in fp16


def _build_program(probe=None):
    nc = bacc.Bacc(
        "TRN2",
        target_bir_lowering=False,
        debug=False,
        enable_asserts=False,
        num_devices=B,
    )
    # x slab rows: 0..1 ones, 2..15 zero pad, 16..63 x (c*16+g)
    x_d = nc.dram_tensor("x", [NSLAB, 64, NFREE], F16, kind="ExternalInput")
    p_d = nc.dram_tensor("p", [NCOLORS, C], F32, kind="ExternalInput")
    out_d = nc.dram_tensor("out", [128, 2], F32, kind="ExternalOutput")

    with tile.TileContext(nc) as tc:
        _body(tc, nc, x_d, p_d, out_d, probe)
    nc.compile()
    return nc


def _body(tc, nc, x_d, p_d, out_d, probe=None):
    import contextlib

    ctx = contextlib.ExitStack()
    const = ctx.enter_context(tc.tile_pool(name="const", bufs=1))
    spool = ctx.enter_context(tc.tile_pool(name="spool", bufs=3))
    mpool = ctx.enter_context(tc.tile_pool(name="mpool", bufs=3))
    cpool = ctx.enter_context(tc.tile_pool(name="cpool", bufs=3))
    s2pool = ctx.enter_context(tc.tile_pool(name="s2pool", bufs=3))
    stpool = ctx.enter_context(tc.tile_pool(name="stpool", bufs=4))
    collp = ctx.enter_context(tc.tile_pool(name="collp", bufs=2))
    sqp = ctx.enter_context(tc.tile_pool(name="sqp", bufs=2))
    zpool = ctx.enter_context(tc.tile_pool(name="zpool", bufs=3, space="PSUM"))
    z2pool = ctx.enter_context(tc.tile_pool(name="z2pool", bufs=1, space="PSUM"))
    z3pool = ctx.enter_context(tc.tile_pool(name="z3pool", bufs=1, space="PSUM"))
    ptpool = ctx.enter_context(tc.tile_pool(name="ptpool", bufs=1, space="PSUM"))

    # ---------------- preamble: constants -------------------------------
    czero = const.tile([128, 1], F32)
    nc.vector.memset(czero, 0.0)
    nc.const_aps.aps[(F32, 0.0)] = czero[:]


    psbt = const.tile([1, C, NCOLORS], F32)
    hp = tc.high_priority()
    hp.__enter__()
    nc.sync.dma_start(out=psbt, in_=p_d.ap().transpose([1, 0]).unsqueeze(0))
    # rhs buffers: rows 0..47 squares, 48..49 ones + 50..63 pad (row 52
    # carries the fp16 printability table in slab 0) + 64..111 x
    rhs_bufs = []
    for i in range(3):
        rhs = const.tile([ROWS, NFREE], F16, tag=f"rhs{i}")
        rhs_bufs.append(rhs)
    for s in range(NSLAB):
        eng = nc.sync if s % 2 == 0 else nc.scalar
        eng.dma_start(out=rhs_bufs[s % 3][ONES0:ROWS, :], in_=x_d.ap()[s])

    # squares all on GPSIMD (Multiply is in its supported op set),
    # in quarters so nothing blocks the Pool stream for long
    def emit_square(s, quarter=None):
        # slab 0 runs on DVE: the Pool list-scheduler orders its stream by
        # its own heuristic and reliably starves the first slab otherwise
        eng = nc.vector if s == 0 else nc.gpsimd
        rhs = rhs_bufs[s % 3]
        qs = [quarter] if quarter is not None else range(4)
        for q in qs:
            q0 = q * (NFREE // 4)
            sl = slice(q0, q0 + NFREE // 4)
            eng.tensor_tensor(
                out=rhs[0:48, sl], in0=rhs[X0:ROWS, sl], in1=rhs[X0:ROWS, sl],
                op=ALU.mult,
            )


    # identity 128x128 fp16 for PE transpose
    iop128 = const.tile([128, 1], I32)
    nc.gpsimd.iota(iop128, pattern=[[0, 1]], base=0, channel_multiplier=1)
    iof128 = const.tile([128, 128], I32)
    nc.gpsimd.iota(iof128, pattern=[[1, 128]], base=0, channel_multiplier=0)
    id128 = const.tile([128, 128], F16)
    nc.vector.tensor_tensor(
        out=id128, in0=iof128, in1=iop128.to_broadcast([128, 128]), op=ALU.is_equal
    )

    # sten[p, g] = ((p & 15) == g); ones rows 48..49 forced to 1 for all g
    iop112 = const.tile([ROWS, 1], I32)
    nc.gpsimd.iota(iop112, pattern=[[0, 1]], base=0, channel_multiplier=1)
    pm112 = const.tile([ROWS, 1], I32)
    nc.vector.tensor_scalar(
        out=pm112, in0=iop112, scalar1=15, scalar2=None, op0=ALU.bitwise_and
    )
    iof16 = const.tile([ROWS, G], I32)
    nc.gpsimd.iota(iof16, pattern=[[1, G]], base=0, channel_multiplier=0)
    sten = const.tile([ROWS, G], F32)
    nc.vector.tensor_tensor(
        out=sten, in0=iof16, in1=pm112.to_broadcast([ROWS, G]), op=ALU.is_equal
    )
    mo_ge = const.tile([ROWS, 1], I32)
    nc.vector.tensor_scalar(out=mo_ge, in0=iop112, scalar1=ONES0 - 1,
                            scalar2=None, op0=ALU.is_gt)
    mo_lt = const.tile([ROWS, 1], I32)
    nc.vector.tensor_scalar(out=mo_lt, in0=iop112, scalar1=ONES0 + 2,
                            scalar2=None, op0=ALU.is_lt)
    mo = const.tile([ROWS, 1], F32)
    nc.vector.tensor_tensor(out=mo, in0=mo_ge, in1=mo_lt, op=ALU.mult)
    nc.vector.tensor_tensor(out=sten, in0=sten,
                            in1=mo.to_broadcast([ROWS, G]), op=ALU.max)

    # ---------------- preamble: weight table ----------------------------
    # q = p - eps;  T_k = |q_k|^2 + eps  (split hi/lo over two fp16 rows)
    qt = const.tile([1, C, NCOLORS], F32)
    nc.vector.tensor_scalar(out=qt, in0=psbt, scalar1=-EPS, scalar2=None,
                            op0=ALU.add)
    q2 = const.tile([1, C, NCOLORS], F32)
    nc.vector.tensor_tensor(out=q2, in0=qt, in1=qt, op=ALU.mult)
    tsum = const.tile([1, NCOLORS], F32)
    nc.vector.tensor_add(out=tsum, in0=q2[:, 0, :], in1=q2[:, 1, :])
    t32 = const.tile([1, NPAD], F32)
    nc.vector.memset(t32, TBIG)
    nc.vector.scalar_tensor_tensor(
        out=t32[:, 0:NCOLORS], in0=tsum, scalar=EPS, in1=q2[:, 2, :],
        op0=ALU.add, op1=ALU.add,
    )
    thi16 = const.tile([1, NPAD], F16)
    nc.vector.tensor_scalar(out=thi16, in0=t32, scalar1=1.0, scalar2=None,
                            op0=ALU.mult)
    thi32 = const.tile([1, NPAD], F32)
    nc.vector.tensor_scalar(out=thi32, in0=thi16, scalar1=1.0, scalar2=None,
                            op0=ALU.mult)
    tlo32 = const.tile([1, NPAD], F32)
    nc.vector.tensor_tensor(out=tlo32, in0=t32, in1=thi32, op=ALU.subtract)

    # wtab [1, 6, 32]: b0 = 1.0 (x^2 rows), b1 = T_hi, b2 = T_lo,
    # b3..b5 = -2 q_c  (padded colors: T = TBIG, q = 0)
    wtab = const.tile([1, 6, NPAD], F32)
    nc.vector.memset(wtab, 0.0)
    nc.vector.memset(wtab[:, 0, :], 1.0)
    nc.vector.tensor_scalar(out=wtab[:, 1, :], in0=thi32, scalar1=1.0,
                            scalar2=None, op0=ALU.mult)
    nc.vector.tensor_scalar(out=wtab[:, 2, :], in0=tlo32, scalar1=1.0,
                            scalar2=None, op0=ALU.mult)
    nc.vector.tensor_scalar(out=wtab[:, 3:6, 0:NCOLORS], in0=qt, scalar1=-2.0,
                            scalar2=None, op0=ALU.mult)

    # broadcast to all partitions, then per-partition-block select
    wbig = const.tile([ROWS, 6 * NPAD], F32)
    nc.gpsimd.partition_broadcast(wbig, wtab.rearrange("p f k -> p (f k)"))
    wsel = const.tile([ROWS, NPAD], F32)
    nc.vector.memset(wsel, 0.0)
    for blk, (lo, hi) in enumerate(
        [(0, 48), (48, 49), (49, 50), (64, 80), (80, 96), (96, 112)]
    ):
        mge = const.tile([ROWS, 1], I32, tag=f"mge{blk}")
        nc.vector.tensor_scalar(
            out=mge, in0=iop112, scalar1=lo - 1, scalar2=None, op0=ALU.is_gt
        )
        mlt = const.tile([ROWS, 1], I32, tag=f"mlt{blk}")
        nc.vector.tensor_scalar(
            out=mlt, in0=iop112, scalar1=hi, scalar2=None, op0=ALU.is_lt
        )
        mm = const.tile([ROWS, 1], I32, tag=f"mm{blk}")
        nc.vector.tensor_tensor(out=mm, in0=mge, in1=mlt, op=ALU.mult)
        nc.vector.copy_predicated(
            out=wsel,
            mask=mm.to_broadcast([ROWS, NPAD]),
            data=wbig[:, blk * NPAD:(blk + 1) * NPAD],
        )

    # lhsT[p, 128j + g*8 + k] = sten[p, g] * wsel[p, 8j + k]   (k minor!)
    lhsT = const.tile([ROWS, NPASS * 128], F16)
    for j in range(NPASS):
        outv = lhsT[:, 128 * j:128 * (j + 1)].rearrange("p (g k) -> p g k", k=CPP)
        in0 = sten.unsqueeze(2).to_broadcast([ROWS, G, CPP])
        in1 = wsel[:, CPP * j:CPP * (j + 1)].unsqueeze(1).to_broadcast(
            [ROWS, G, CPP])
        nc.vector.tensor_tensor(out=outv, in0=in0, in1=in1, op=ALU.mult)
    emit_square(0)
    emit_square(1)
    hp.__exit__(None, None, None)




    z2 = z2pool.tile([128, 2, MMN], F32)       # pass 2, parity slots
    z3 = z3pool.tile([128, 2, MMN], F32)       # pass 3, parity slots
    pt = ptpool.tile([128, 2, 4, 128], F16)    # transposed survivors

    acc = const.tile([128, 2], F32)
    if probe is not None:
        nc.vector.memset(acc, 0.0)

    # PE p-state warm-up: harmless matmuls on the weight tile ramp the
    # tensor engine to full clock just before the first real matmuls
    for _ in range(12):
        zw = zpool.tile([128, MMN], F32, tag="z01")
        nc.tensor.matmul(out=zw, lhsT=lhsT[:, 0:128], rhs=lhsT[:, 0:MMN],
                         start=True, stop=True)

    collectors = []

    def emit_color_min(pair, stile_of):
        # transposes (PE) for `pair`, then fold the packed 8 colors (DVE)
        stile = stile_of[pair]
        for par in range(2):
            for chb in range(4):
                nc.tensor.transpose(
                    out=pt[:, par, chb, :],
                    in_=stile[:, par, 128 * chb:128 * (chb + 1)],
                    identity=id128,
                )
        if pair % 8 == 0:
            coll_new = collp.tile([128, 8, 128], F16, tag="coll")
            collectors.append(coll_new)
        coll = collectors[-1]
        ptv = pt.rearrange("p q c (g k) -> p q c g k", k=CPP)
        outv = coll[:, pair % 8, :].rearrange("p (q c g) -> p q c g", q=2, c=4)
        nc.vector.tensor_reduce(
            out=outv, in_=ptv, axis=mybir.AxisListType.X, op=ALU.min
        )

    def emit_collector_finish(r):
        coll = collectors[r]
        nc.vector.tensor_scalar(
            out=coll, in0=coll, scalar1=0.0, scalar2=None, op0=ALU.max
        )
        scratch = sqp.tile([128, 8 * 128], F16, tag="sq")
        nc.scalar.activation(
            out=scratch, in_=coll.rearrange("p a b -> p (a b)"),
            func=ACTF.Sqrt, accum_out=acc[:, r:r + 1],
        )

    # ---------------- main loop -----------------------------------------
    stile_of = {}
    for pair in range(NPAIR):
        slab = pair // 4
        rhs = rhs_bufs[slab % 3]
        s16 = spool.tile([128, 2, 2, MMN], F16, tag="s16")
        for par in range(2):
            st = pair * 2 + par
            t = st % STS
            rsl = rhs[:, t * MMN:(t + 1) * MMN]
            for j in range(NPASS):
                if j < 2:
                    zt = zpool.tile([128, MMN], F32, tag="z01")
                elif j == 2:
                    zt = z2[:, par, :]
                else:
                    zt = z3[:, par, :]
                nc.tensor.matmul(
                    out=zt,
                    lhsT=lhsT[:, 128 * j:128 * (j + 1)],
                    rhs=rsl,
                    start=True,
                    stop=True,
                )
                if j < 2:
                    nc.scalar.activation(
                        out=s16[:, par, j, :], in_=zt, func=ACTF.Relu
                    )
        # transposes + color fold of an older pair ride here so the PE
        # never waits on the (deep) min pipeline
        if probe != "pe_only" and pair >= 3:
            emit_color_min(pair - 3, stile_of)
            stile_of.pop(pair - 3)

        if probe == "pe_only":
            continue

        # ScalarE also converts pass 2 (pair-merged); DVE folds the three
        # converted passes at fp16 2x and chains pass 3 from PSUM
        s2 = s2pool.tile([128, 2, MMN], F16, tag="s2")
        nc.scalar.activation(out=s2, in_=z2, func=ACTF.Relu)
        m1 = mpool.tile([128, 2, MMN], F16, tag="m1")
        nc.vector.tensor_tensor(
            out=m1, in0=s16[:, :, 0, :], in1=s16[:, :, 1, :], op=ALU.min
        )
        m2 = cpool.tile([128, 2, MMN], F16, tag="m2")
        nc.vector.tensor_tensor(out=m2, in0=m1, in1=s2, op=ALU.min)
        stile = stpool.tile([128, 2, MMN], F16, tag="stile")
        nc.vector.tensor_tensor(out=stile, in0=m2, in1=z3, op=ALU.min)
        stile_of[pair] = stile

        if 3 <= pair <= 6:
            emit_square(2, quarter=pair - 3)
        elif 7 <= pair <= 10:
            emit_square(3, quarter=pair - 7)


    if probe != "pe_only":
        for p in (NPAIR - 3, NPAIR - 2, NPAIR - 1):
            emit_color_min(p, stile_of)
        emit_collector_finish(0)
        emit_collector_finish(1)

    nc.sync.dma_start(out=out_d.ap(), in_=acc)
    ctx.close()


_CACHE = {}


def _get_program(probe=None):
    key = ("prog", probe)
    if key not in _CACHE:
        _CACHE[key] = _build_program(probe)
    return _CACHE[key]


def _prep_x(adv_patch):
    # device layout per slab: rows 0..1 ones, 2..15 zero, 16..63 x(c*16+g)
    x = (
        np.asarray(adv_patch, dtype=np.float32)
        .reshape(B, C, NSLAB, G, NFREE)
        .transpose(0, 2, 1, 3, 4)
        .reshape(B, NSLAB, 48, NFREE)
        .astype(np.float16)
    )
    xd = np.zeros((B, NSLAB, 64, NFREE), dtype=np.float16)
    xd[:, :, 0:2, :] = np.float16(1.0)
    xd[:, :, 16:64, :] = x
    return np.ascontiguousarray(xd)


def kernel(adv_patch: np.ndarray, printability: np.ndarray) -> np.ndarray:
    xd = _prep_x(adv_patch)
    p = np.ascontiguousarray(printability, dtype=np.float32)
    nc = _get_program()
    in_maps = [{"x": xd[b], "p": p} for b in range(B)]
    res = run_bass_kernel_spmd(nc, in_maps, core_ids=list(range(B)))
    total = np.float64(0.0)
    for r in res.results:
        total += r["out"].astype(np.float64).sum()
    return np.float32(total / (B * C * H * W))


def profile_once(inputs, trace_cores=None):
    xd = _prep_x(inputs["adv_patch"])
    p = np.ascontiguousarray(inputs["printability"], dtype=np.float32)
    nc = _get_program()
    in_maps = [{"x": xd[b], "p": p} for b in range(B)]
    try:
        res = run_bass_kernel_spmd(
            nc, in_maps, core_ids=list(range(B)), trace=True,
            trace_cores=trace_cores,
        )
        if res.instructions_and_trace is not None:
            print("trace:", res.instructions_and_trace[1])
        return res.exec_time_ns
    except Exception as e:
        print("profile_once failed:", e)
        return None


# revision 22
# speedup vs baseline: 1.7199x; 1.0340x over previous
"""NPS (non-printability score) kernel for Trainium2, 8-core data-parallel.

Math: for each pixel x (3 channels), distance to each of 30 printability
colors p_k is  d2_k = sum_c (x_c - p_c + 1e-6)^2 + 1e-6.  The score is
sum over pixels of sqrt(min_k d2_k), divided by adv_patch.size.

With q = p - 1e-6:  d2_k = S + (-2 x.q_k) + (T_k + 1e-6) where S = sum
x_c^2, T_k = |q_k|^2.  fp16 matmuls (1 PE cycle/column vs fp32's 4)
compute d2 for 8 colors x 16 pixel groups per 512-column pass; 4 passes
cover the 32 (padded) colors.  T rides in as two "ones"-row weights
(split hi/lo over two fp16 rows to kill weight-rounding error); the
ones rows and x rows arrive in a single HBM load per slab.

Post-matmul min funnel (z is fp32 in PSUM - TRN2 matmuls can't narrow -
so every z value crosses PSUM->SBUF exactly once, spread over engines):
  - ScalarE: passes 0,1 -> fp16 SBUF via Relu (per-slot ops on a 3-bank
    rotation so PSUM frees early; Relu also clamps rounding negatives).
  - DVE: fp16 min of the two converted passes (packed fp16 = 2x rate),
    then a chained min against pass 2 read straight from PSUM.
  - Pass 3 is DMA-drained to SBUF so GPSIMD - which cannot touch PSUM -
    applies the last pass-min.
  - PE transposes the survivor (fp16 2x; colors k are the innermost 8
    of each transposed block since lhsT columns are g*8+k), then one
    DVE windowed reduce_min folds the colors; per-pixel minima collect
    into a [128, 8*128] tile per 8 pairs; a tensor_scalar max-0 clamp
    (fp16 rounding can push min-d2 slightly negative) and one ScalarE
    sqrt+accumulate per collector finish the job.

PSUM (8 banks): z01 rotation 3 + z2 [128,2,512] 2 + z3 stage 2 + pt 1.

Sharding: batch dim (8 images) -> 8 NeuronCores, printability replicated.
"""

import numpy as np

import concourse.bass as bass
import concourse.bacc as bacc
import concourse.tile as tile
import concourse.mybir as mybir
from concourse.bass_utils import run_bass_kernel_spmd

F32 = mybir.dt.float32
F16 = mybir.dt.float16
I32 = mybir.dt.int32
ALU = mybir.AluOpType
ACTF = mybir.ActivationFunctionType

B, C, H, W = 8, 3, 512, 512
NCOLORS = 30
NPAD = 32            # colors padded to 32
NPASS = 4            # color passes, 8 colors each
CPP = 8              # colors per pass
G = 16               # pixel groups per matmul column block
MMN = 512            # matmul moving free dim (one fp32 PSUM bank)
NFREE = 4096         # per-partition free size of one slab
NSLAB = 4            # 4 slabs x 16 groups x 4096 = 262144 pixels/core
STS = NFREE // MMN   # supertiles per slab = 8
NPAIR = NSLAB * STS // 2   # 16 parity pairs
X0 = 64              # x rows base in rhs (rows 50..63 are zero pad)
ONES0 = 48           # two ones rows (T_hi / T_lo weights)
ROWS = 112
EPS = 1e-6
TBIG = 60000.0       # padded-color T: huge but finite in fp16


def _build_program(probe=None):
    nc = bacc.Bacc(
        "TRN2",
        target_bir_lowering=False,
        debug=False,
        enable_asserts=False,
        num_devices=B,
    )
    # x slab rows: 0..1 ones, 2..15 zero pad, 16..63 x (c*16+g)
    x_d = nc.dram_tensor("x", [NSLAB, 64, NFREE], F16, kind="ExternalInput")
    p_d = nc.dram_tensor("p", [NCOLORS, C], F32, kind="ExternalInput")
    out_d = nc.dram_tensor("out", [128, 2], F32, kind="ExternalOutput")

    with tile.TileContext(nc) as tc:
        _body(tc, nc, x_d, p_d, out_d, probe)
    nc.compile()
    return nc


def _body(tc, nc, x_d, p_d, out_d, probe=None):
    import contextlib

    ctx = contextlib.ExitStack()
    const = ctx.enter_context(tc.tile_pool(name="const", bufs=1))
    spool = ctx.enter_context(tc.tile_pool(name="spool", bufs=3))
    mpool = ctx.enter_context(tc.tile_pool(name="mpool", bufs=3))
    cpool = ctx.enter_context(tc.tile_pool(name="cpool", bufs=3))
    s2pool = ctx.enter_context(tc.tile_pool(name="s2pool", bufs=3))
    stpool = ctx.enter_context(tc.tile_pool(name="stpool", bufs=4))
    collp = ctx.enter_context(tc.tile_pool(name="collp", bufs=2))
    t1pool = ctx.enter_context(tc.tile_pool(name="t1pool", bufs=2))
    sqp = ctx.enter_context(tc.tile_pool(name="sqp", bufs=2))
    zpool = ctx.enter_context(tc.tile_pool(name="zpool", bufs=3, space="PSUM"))
    z2pool = ctx.enter_context(tc.tile_pool(name="z2pool", bufs=1, space="PSUM"))
    z3pool = ctx.enter_context(tc.tile_pool(name="z3pool", bufs=1, space="PSUM"))
    ptpool = ctx.enter_context(tc.tile_pool(name="ptpool", bufs=1, space="PSUM"))

    # ---------------- preamble: constants -------------------------------
    czero = const.tile([128, 1], F32)
    nc.vector.memset(czero, 0.0)
    nc.const_aps.aps[(F32, 0.0)] = czero[:]


    psbt = const.tile([1, C, NCOLORS], F32)
    hp = tc.high_priority()
    hp.__enter__()
    nc.sync.dma_start(out=psbt, in_=p_d.ap().transpose([1, 0]).unsqueeze(0))
    # rhs buffers: rows 0..47 squares, 48..49 ones + 50..63 pad (row 52
    # carries the fp16 printability table in slab 0) + 64..111 x
    rhs_bufs = []
    for i in range(3):
        rhs = const.tile([ROWS, NFREE], F16, tag=f"rhs{i}")
        rhs_bufs.append(rhs)
    for s in range(NSLAB):
        eng = nc.sync if s % 2 == 0 else nc.scalar
        eng.dma_start(out=rhs_bufs[s % 3][ONES0:ROWS, :], in_=x_d.ap()[s])

    # squares all on GPSIMD (Multiply is in its supported op set),
    # in quarters so nothing blocks the Pool stream for long
    def emit_square(s, quarter=None):
        # slab 0 runs on DVE: the Pool list-scheduler orders its stream by
        # its own heuristic and reliably starves the first slab otherwise
        eng = nc.vector if s == 0 else nc.gpsimd
        rhs = rhs_bufs[s % 3]
        qs = [quarter] if quarter is not None else range(4)
        for q in qs:
            q0 = q * (NFREE // 4)
            sl = slice(q0, q0 + NFREE // 4)
            eng.tensor_tensor(
                out=rhs[0:48, sl], in0=rhs[X0:ROWS, sl], in1=rhs[X0:ROWS, sl],
                op=ALU.mult,
            )


    # identity 128x128 fp16 for PE transpose
    iop128 = const.tile([128, 1], I32)
    nc.gpsimd.iota(iop128, pattern=[[0, 1]], base=0, channel_multiplier=1)
    iof128 = const.tile([128, 128], I32)
    nc.gpsimd.iota(iof128, pattern=[[1, 128]], base=0, channel_multiplier=0)
    id128 = const.tile([128, 128], F16)
    nc.vector.tensor_tensor(
        out=id128, in0=iof128, in1=iop128.to_broadcast([128, 128]), op=ALU.is_equal
    )

    # sten[p, g] = ((p & 15) == g); ones rows 48..49 forced to 1 for all g
    iop112 = const.tile([ROWS, 1], I32)
    nc.gpsimd.iota(iop112, pattern=[[0, 1]], base=0, channel_multiplier=1)
    pm112 = const.tile([ROWS, 1], I32)
    nc.vector.tensor_scalar(
        out=pm112, in0=iop112, scalar1=15, scalar2=None, op0=ALU.bitwise_and
    )
    iof16 = const.tile([ROWS, G], I32)
    nc.gpsimd.iota(iof16, pattern=[[1, G]], base=0, channel_multiplier=0)
    sten = const.tile([ROWS, G], F32)
    nc.vector.tensor_tensor(
        out=sten, in0=iof16, in1=pm112.to_broadcast([ROWS, G]), op=ALU.is_equal
    )
    mo_ge = const.tile([ROWS, 1], I32)
    nc.vector.tensor_scalar(out=mo_ge, in0=iop112, scalar1=ONES0 - 1,
                            scalar2=None, op0=ALU.is_gt)
    mo_lt = const.tile([ROWS, 1], I32)
    nc.vector.tensor_scalar(out=mo_lt, in0=iop112, scalar1=ONES0 + 2,
                            scalar2=None, op0=ALU.is_lt)
    mo = const.tile([ROWS, 1], F32)
    nc.vector.tensor_tensor(out=mo, in0=mo_ge, in1=mo_lt, op=ALU.mult)
    nc.vector.tensor_tensor(out=sten, in0=sten,
                            in1=mo.to_broadcast([ROWS, G]), op=ALU.max)

    # ---------------- preamble: weight table ----------------------------
    # q = p - eps;  T_k = |q_k|^2 + eps  (split hi/lo over two fp16 rows)
    qt = const.tile([1, C, NCOLORS], F32)
    nc.vector.tensor_scalar(out=qt, in0=psbt, scalar1=-EPS, scalar2=None,
                            op0=ALU.add)
    q2 = const.tile([1, C, NCOLORS], F32)
    nc.vector.tensor_tensor(out=q2, in0=qt, in1=qt, op=ALU.mult)
    tsum = const.tile([1, NCOLORS], F32)
    nc.vector.tensor_add(out=tsum, in0=q2[:, 0, :], in1=q2[:, 1, :])
    t32 = const.tile([1, NPAD], F32)
    nc.vector.memset(t32, TBIG)
    nc.vector.scalar_tensor_tensor(
        out=t32[:, 0:NCOLORS], in0=tsum, scalar=EPS, in1=q2[:, 2, :],
        op0=ALU.add, op1=ALU.add,
    )
    thi16 = const.tile([1, NPAD], F16)
    nc.vector.tensor_scalar(out=thi16, in0=t32, scalar1=1.0, scalar2=None,
                            op0=ALU.mult)
    thi32 = const.tile([1, NPAD], F32)
    nc.vector.tensor_scalar(out=thi32, in0=thi16, scalar1=1.0, scalar2=None,
                            op0=ALU.mult)
    tlo32 = const.tile([1, NPAD], F32)
    nc.vector.tensor_tensor(out=tlo32, in0=t32, in1=thi32, op=ALU.subtract)

    # wtab [1, 6, 32]: b0 = 1.0 (x^2 rows), b1 = T_hi, b2 = T_lo,
    # b3..b5 = -2 q_c  (padded colors: T = TBIG, q = 0)
    wtab = const.tile([1, 6, NPAD], F32)
    nc.vector.memset(wtab, 0.0)
    nc.vector.memset(wtab[:, 0, :], 1.0)
    nc.vector.tensor_scalar(out=wtab[:, 1, :], in0=thi32, scalar1=1.0,
                            scalar2=None, op0=ALU.mult)
    nc.vector.tensor_scalar(out=wtab[:, 2, :], in0=tlo32, scalar1=1.0,
                            scalar2=None, op0=ALU.mult)
    nc.vector.tensor_scalar(out=wtab[:, 3:6, 0:NCOLORS], in0=qt, scalar1=-2.0,
                            scalar2=None, op0=ALU.mult)

    # broadcast to all partitions, then per-partition-block select
    wbig = const.tile([ROWS, 6 * NPAD], F32)
    nc.gpsimd.partition_broadcast(wbig, wtab.rearrange("p f k -> p (f k)"))
    wsel = const.tile([ROWS, NPAD], F32)
    nc.vector.memset(wsel, 0.0)
    for blk, (lo, hi) in enumerate(
        [(0, 48), (48, 49), (49, 50), (64, 80), (80, 96), (96, 112)]
    ):
        mge = const.tile([ROWS, 1], I32, tag=f"mge{blk}")
        nc.vector.tensor_scalar(
            out=mge, in0=iop112, scalar1=lo - 1, scalar2=None, op0=ALU.is_gt
        )
        mlt = const.tile([ROWS, 1], I32, tag=f"mlt{blk}")
        nc.vector.tensor_scalar(
            out=mlt, in0=iop112, scalar1=hi, scalar2=None, op0=ALU.is_lt
        )
        mm = const.tile([ROWS, 1], I32, tag=f"mm{blk}")
        nc.vector.tensor_tensor(out=mm, in0=mge, in1=mlt, op=ALU.mult)
        nc.vector.copy_predicated(
            out=wsel,
            mask=mm.to_broadcast([ROWS, NPAD]),
            data=wbig[:, blk * NPAD:(blk + 1) * NPAD],
        )

    # lhsT[p, 128j + g*8 + k] = sten[p, g] * wsel[p, 8j + k]   (k minor!)
    lhsT = const.tile([ROWS, NPASS * 128], F16)
    for j in range(NPASS):
        outv = lhsT[:, 128 * j:128 * (j + 1)].rearrange("p (g k) -> p g k", k=CPP)
        in0 = sten.unsqueeze(2).to_broadcast([ROWS, G, CPP])
        in1 = wsel[:, CPP * j:CPP * (j + 1)].unsqueeze(1).to_broadcast(
            [ROWS, G, CPP])
        nc.vector.tensor_tensor(out=outv, in0=in0, in1=in1, op=ALU.mult)
    emit_square(0)
    emit_square(1)
    hp.__exit__(None, None, None)




    z2 = z2pool.tile([128, 2, MMN], F32)       # pass 2, parity slots
    z3 = z3pool.tile([128, 2, MMN], F32)       # pass 3, parity slots
    pt = ptpool.tile([128, 2, 4, 128], F16)    # transposed survivors

    acc = const.tile([128, 2], F32)
    if probe is not None:
        nc.vector.memset(acc, 0.0)

    # PE p-state warm-up: harmless matmuls on the weight tile ramp the
    # tensor engine to full clock just before the first real matmuls
    for _ in range(12):
        zw = zpool.tile([128, MMN], F32, tag="z01")
        nc.tensor.matmul(out=zw, lhsT=lhsT[:, 0:128], rhs=lhsT[:, 0:MMN],
                         start=True, stop=True)

    collectors = []

    def emit_color_min(pair, stile_of):
        # transposes (PE) for `pair`, then fold the packed 8 colors (DVE)
        stile = stile_of[pair]
        for par in range(2):
            for chb in range(4):
                nc.tensor.transpose(
                    out=pt[:, par, chb, :],
                    in_=stile[:, par, 128 * chb:128 * (chb + 1)],
                    identity=id128,
                )
        if pair % 8 == 0:
            coll_new = collp.tile([128, 8, 128], F16, tag="coll")
            collectors.append(coll_new)
        coll = collectors[-1]
        ptv = pt.rearrange("p q c (g k) -> p q c g k", k=CPP)
        outv = coll[:, pair % 8, :].rearrange("p (q c g) -> p q c g", q=2, c=4)
        # packed fp16 min tree: one PSUM half is copied out first so every
        # TT sees at most one PSUM operand; packed fp16 runs at 2x
        u = t1pool.tile([128, 2, 4, G, 4], F16, tag="u")
        nc.vector.tensor_copy(out=u, in_=ptv[:, :, :, :, 4:8])
        t1 = t1pool.tile([128, 2, 4, G, 4], F16, tag="t1")
        nc.vector.tensor_tensor(out=t1, in0=ptv[:, :, :, :, 0:4], in1=u,
                                op=ALU.min)
        t2 = t1pool.tile([128, 2, 4, G, 2], F16, tag="t2")
        nc.vector.tensor_tensor(out=t2, in0=t1[:, :, :, :, 0:2],
                                in1=t1[:, :, :, :, 2:4], op=ALU.min)
        nc.vector.tensor_tensor(out=outv, in0=t2[:, :, :, :, 0],
                                in1=t2[:, :, :, :, 1], op=ALU.min)

    def emit_collector_finish(r):
        coll = collectors[r]
        nc.vector.tensor_scalar(
            out=coll, in0=coll, scalar1=0.0, scalar2=None, op0=ALU.max
        )
        scratch = sqp.tile([128, 8 * 128], F16, tag="sq")
        nc.scalar.activation(
            out=scratch, in_=coll.rearrange("p a b -> p (a b)"),
            func=ACTF.Sqrt, accum_out=acc[:, r:r + 1],
        )

    # ---------------- main loop -----------------------------------------
    stile_of = {}
    for pair in range(NPAIR):
        slab = pair // 4
        rhs = rhs_bufs[slab % 3]
        s16 = spool.tile([128, 2, 2, MMN], F16, tag="s16")
        for par in range(2):
            st = pair * 2 + par
            t = st % STS
            rsl = rhs[:, t * MMN:(t + 1) * MMN]
            for j in range(NPASS):
                if j < 2:
                    zt = zpool.tile([128, MMN], F32, tag="z01")
                elif j == 2:
                    zt = z2[:, par, :]
                else:
                    zt = z3[:, par, :]
                nc.tensor.matmul(
                    out=zt,
                    lhsT=lhsT[:, 128 * j:128 * (j + 1)],
                    rhs=rsl,
                    start=True,
                    stop=True,
                )
                if j < 2:
                    nc.scalar.activation(
                        out=s16[:, par, j, :], in_=zt, func=ACTF.Relu
                    )
        # transposes + color fold of an older pair ride here so the PE
        # never waits on the (deep) min pipeline
        if probe != "pe_only" and pair >= 3:
            emit_color_min(pair - 3, stile_of)
            stile_of.pop(pair - 3)

        if probe == "pe_only":
            continue

        # ScalarE also converts pass 2 (pair-merged); DVE folds the three
        # converted passes at fp16 2x and chains pass 3 from PSUM
        s2 = s2pool.tile([128, 2, MMN], F16, tag="s2")
        nc.scalar.activation(out=s2, in_=z2, func=ACTF.Relu)
        m1 = mpool.tile([128, 2, MMN], F16, tag="m1")
        nc.vector.tensor_tensor(
            out=m1, in0=s16[:, :, 0, :], in1=s16[:, :, 1, :], op=ALU.min
        )
        m2 = cpool.tile([128, 2, MMN], F16, tag="m2")
        nc.vector.tensor_tensor(out=m2, in0=m1, in1=s2, op=ALU.min)
        stile = stpool.tile([128, 2, MMN], F16, tag="stile")
        nc.vector.tensor_tensor(out=stile, in0=m2, in1=z3, op=ALU.min)
        stile_of[pair] = stile

        if 3 <= pair <= 6:
            emit_square(2, quarter=pair - 3)
        elif 7 <= pair <= 10:
            emit_square(3, quarter=pair - 7)


    if probe != "pe_only":
        for p in (NPAIR - 3, NPAIR - 2, NPAIR - 1):
            emit_color_min(p, stile_of)
        emit_collector_finish(0)
        emit_collector_finish(1)

    nc.sync.dma_start(out=out_d.ap(), in_=acc)
    ctx.close()


_CACHE = {}


def _get_program(probe=None):
    key = ("prog", probe)
    if key not in _CACHE:
        _CACHE[key] = _build_program(probe)
    return _CACHE[key]


def _prep_x(adv_patch):
    # device layout per slab: rows 0..1 ones, 2..15 zero, 16..63 x(c*16+g)
    x = (
        np.asarray(adv_patch, dtype=np.float32)
        .reshape(B, C, NSLAB, G, NFREE)
        .transpose(0, 2, 1, 3, 4)
        .reshape(B, NSLAB, 48, NFREE)
        .astype(np.float16)
    )
    xd = np.zeros((B, NSLAB, 64, NFREE), dtype=np.float16)
    xd[:, :, 0:2, :] = np.float16(1.0)
    xd[:, :, 16:64, :] = x
    return np.ascontiguousarray(xd)


def kernel(adv_patch: np.ndarray, printability: np.ndarray) -> np.ndarray:
    xd = _prep_x(adv_patch)
    p = np.ascontiguousarray(printability, dtype=np.float32)
    nc = _get_program()
    in_maps = [{"x": xd[b], "p": p} for b in range(B)]
    res = run_bass_kernel_spmd(nc, in_maps, core_ids=list(range(B)))
    total = np.float64(0.0)
    for r in res.results:
        total += r["out"].astype(np.float64).sum()
    return np.float32(total / (B * C * H * W))


def profile_once(inputs, trace_cores=None):
    xd = _prep_x(inputs["adv_patch"])
    p = np.ascontiguousarray(inputs["printability"], dtype=np.float32)
    nc = _get_program()
    in_maps = [{"x": xd[b], "p": p} for b in range(B)]
    try:
        res = run_bass_kernel_spmd(
            nc, in_maps, core_ids=list(range(B)), trace=True,
            trace_cores=trace_cores,
        )
        if res.instructions_and_trace is not None:
            print("trace:", res.instructions_and_trace[1])
        return res.exec_time_ns
    except Exception as e:
        print("profile_once failed:", e)
        return None
